# revision 23
# baseline (speedup 1.0000x reference)
"""Trainium2 Bass kernel for nn_CABlock (channel attention / XCA block).

Reference computation (per batch b):
  qkv = x @ qkv_w.T                      # [N, 3C], token-major
  q,k,v per head: [d=64, N] channel-major after reshape/transpose
  q,k l2-normalized over N; attn = softmax((q @ k.T) * temperature, axis=-1)
  out = attn @ v  -> [N, C];  y = out @ proj_w.T + proj_b

Key algebraic restructure: l2norm commutes with the bilinear form, so
  attn_logits = diag(inv_q) @ (q_raw @ k_raw.T) @ diag(inv_k) * temp
with inv_q[c] = 1/max(||q[c,:]||,eps). We accumulate q^T k Gram blocks and
per-channel sums of squares (via a ones-vector matmul over squared values)
in PSUM across all token chunks, then apply the tiny [64x64]-per-head
softmax at the end. This avoids ever materializing normalized q/k.

Sharding: data-parallel over batch B=16 across 8 cores (2 batches/core).
No collectives needed.
"""

import os
import sys

import numpy as np

for _p in ("/opt/trn_rl_repo", "/root/.axon_site/_ro/trn_rl_repo"):
    if os.path.isdir(_p) and _p not in sys.path:
        sys.path.insert(0, _p)

import concourse.bass as bass  # noqa: E402
from concourse import mybir  # noqa: E402
from concourse.bass import ts  # noqa: E402
from concourse.bass_utils import run_bass_kernel_spmd  # noqa: E402
from concourse.masks import make_identity  # noqa: E402
from concourse.tile import TileContext  # noqa: E402

B, N, C = 16, 4096, 512
H, D = 8, 64
C3 = 3 * C
NCORES = 8
BL = B // NCORES  # batches per core
EPS = 1e-12
NCHUNK = N // 128  # 32 token chunks per batch
F32 = mybir.dt.float32

# Matmul operand mode: "bf16" | "fp32" | "f32r" (set BASS_MM_MODE to override)
MM_MODE = os.environ.get("BASS_MM_MODE", "bf16")


def legalize_waits(nc):
    """Walrus in this environment rejects instructions carrying more than one
    semaphore wait ("Too many sync wait commands"), and rejects sem-ge waits
    on Drain instructions entirely. Tile emits both. Hoist the offending
    waits onto standalone EventSemaphore instructions inserted immediately
    before the instruction on the same engine queue — semantically identical
    (the engine executes the waits, then the instruction)."""
    n_new = 0
    for bb in nc.main_func.blocks:
        il = bb.instructions
        new_list = []
        for ins in il:
            si = ins.sync_info
            waits = list(si.on_wait) if si is not None and si.on_wait else []
            if waits:
                is_drain = type(ins).__name__ == "InstDrain" or (
                    getattr(ins, "opcode", "") == "Drain"
                )
                keep_budget = 0 if is_drain else 1
                if len(waits) > keep_budget:
                    hoist, keep = waits[:-keep_budget] if keep_budget else waits, (
                        waits[-keep_budget:] if keep_budget else []
                    )
                    for w in hoist:
                        ev = mybir.InstEventSemaphore(
                            name=f"{ins.name}-hoistw{n_new}",
                            ins=[],
                            outs=[],
                            engine=ins.engine,
                            sync_info=mybir.SyncInfo(on_wait=[w], on_update=[]),
                        )
                        new_list.append(ev)
                        n_new += 1
                    ins.sync_info = mybir.SyncInfo(
                        on_wait=keep, on_update=list(si.on_update or [])
                    )
            new_list.append(ins)
        il.clear()
        il.extend(new_list)
    return n_new


def build_bass():
    mode = MM_MODE
    op_dt = mybir.dt.bfloat16 if mode == "bf16" else F32

    def mm(ap):
        """Cast an operand AP at a matmul call site for the big matmuls."""
        if mode == "f32r":
            return ap.bitcast(mybir.dt.float32r)
        return ap

    nc = bass.Bass(trn_type="TRN2")
    x = nc.dram_tensor("x", [BL, N, C], F32, kind="ExternalInput")
    qkv_w = nc.dram_tensor("qkv_w", [C3, C], F32, kind="ExternalInput")
    temp = nc.dram_tensor("temperature", [H], F32, kind="ExternalInput")
    proj_w = nc.dram_tensor("proj_w", [C, C], F32, kind="ExternalInput")
    proj_b = nc.dram_tensor("proj_b", [C], F32, kind="ExternalInput")
    out = nc.dram_tensor("out", [BL, N, C], F32, kind="ExternalOutput")

    with TileContext(nc) as tc:
        consts = tc.alloc_tile_pool(name="consts", bufs=1)
        wstage = tc.alloc_tile_pool(name="wstage", bufs=2)
        xin = tc.alloc_tile_pool(name="xin", bufs=3)
        chunk = tc.alloc_tile_pool(name="chunk", bufs=2)
        # fp32 modes double the vT footprint; drop cross-batch double-buffering
        vtp = tc.alloc_tile_pool(name="vtp", bufs=8 if op_dt != F32 else 4)
        small = tc.alloc_tile_pool(name="small", bufs=2)
        outp = tc.alloc_tile_pool(name="outp", bufs=2)
        yp = tc.alloc_tile_pool(name="yp", bufs=3)
        ps = tc.alloc_tile_pool(name="ps", bufs=5, space="PSUM")
        accp = tc.alloc_tile_pool(name="accp", bufs=1, space="PSUM")

        # ---- constants ----
        ident = consts.tile([128, 128], op_dt)
        make_identity(nc, ident)
        ones_col = consts.tile([128, 1], op_dt)
        nc.vector.memset(ones_col, 1.0)
        ones_f32 = consts.tile([1, 128], F32)
        nc.vector.memset(ones_f32, 1.0)
        id1_f32 = consts.tile([1, 1], F32)
        nc.vector.memset(id1_f32, 1.0)

        # temperature: [1, H] row (broadcast along D later via 0-step APs)
        temp_sb = consts.tile([1, H], F32)
        nc.sync.dma_start(out=temp_sb, in_=temp[:])

        # proj bias: load [1, C] then broadcast to all partitions via a
        # K=1 ones-matmul on the PE (out[p, c] = 1 * bias[c])
        bias_row = consts.tile([1, C], F32)
        nc.sync.dma_start(out=bias_row, in_=proj_b[:])
        bias_ps = ps.tile([128, C], F32, tag="ps")
        nc.tensor.matmul(bias_ps, ones_f32, bias_row, start=True, stop=True)
        bias_bc = consts.tile([128, C], F32)
        nc.vector.tensor_copy(out=bias_bc, in_=bias_ps)

        # ---- weights: load natural layout, transpose on PE into [cin, cout] ----
        # qkv_wT[kc] : [128 (cin chunk kc), C3 (cout)]
        qkv_wT = [consts.tile([128, C3], op_dt, tag=f"qkvwT{i}", name=f"qkvwT{i}") for i in range(4)]
        proj_wT = [consts.tile([128, C], op_dt, tag=f"projwT{i}", name=f"projwT{i}") for i in range(4)]

        def load_transposed_weight(w_dram, n_row_tiles, dst_tiles):
            # w_dram: [rows, C] -> dst_tiles[kc][:, mc*128:(mc+1)*128] = w.T chunks
            for mc in range(n_row_tiles):
                wnat = wstage.tile([128, C], F32, tag="wnat")
                nc.sync.dma_start(out=wnat, in_=w_dram[ts(mc, 128), :])
                if op_dt != F32:
                    wb = wstage.tile([128, C], op_dt, tag="wb")
                    nc.gpsimd.tensor_copy(out=wb, in_=wnat)
                else:
                    wb = wnat
                wtp = ps.tile([128, 4, 128], op_dt, tag="ps")
                for kc in range(4):
                    nc.tensor.transpose(wtp[:, kc, :], wb[:, ts(kc, 128)], ident)
                for kc in range(4):
                    nc.vector.tensor_copy(
                        out=dst_tiles[kc][:, ts(mc, 128)], in_=wtp[:, kc, :]
                    )

        load_transposed_weight(qkv_w, C3 // 128, qkv_wT)
        load_transposed_weight(proj_w, C // 128, proj_wT)

        for b in range(BL):
            # Persistent per-batch PSUM accumulators. Heads are "pair-packed":
            # head h lives at partitions (h%2)*64..(h%2)*64+63, pair slot h//2.
            #   acc[(h%2)*64:+64, (h//2)*64:+64] = sum_n q_h[n,:]^T k_h[n,:]
            #   ssq_q[0:1, :] = sum_n q[n,c]^2 ; ssq_k[0:1, :] = sum_n k[n,c]^2
            acc = accp.tile([128, 4, D], F32, tag="acc")
            ssq_q = accp.tile([1, C], F32, tag="ssq_q")
            ssq_k = accp.tile([1, C], F32, tag="ssq_k")
            # v in channel-major layout, 4 channel-group tiles [128, N]
            vT = [vtp.tile([128, N], op_dt, tag="vt", name=f"vt{g}") for g in range(4)]

            # ---------------- Phase A: stream token chunks ----------------
            for ci in range(NCHUNK):
                xt = xin.tile([128, C], F32, tag="xt")
                nc.sync.dma_start(out=xt, in_=x[b, ts(ci, 128), :])
                if op_dt != F32:
                    xb = chunk.tile([128, C], op_dt, tag="xb")
                    nc.gpsimd.tensor_copy(out=xb, in_=xt)
                else:
                    xb = xt

                # transpose x chunk: [128 tok, C] -> [C, 128 tok] (4 blocks)
                xtp = ps.tile([128, 4, 128], op_dt, tag="ps")
                for g in range(4):
                    nc.tensor.transpose(xtp[:, g, :], xb[:, ts(g, 128)], ident)
                xts = chunk.tile([128, 4, 128], op_dt, tag="xts")
                nc.vector.tensor_copy(out=xts, in_=xtp)

                # qkv projection, token-major: [128 tok, C3] in 3 psum groups
                qp = ps.tile([128, C], F32, tag="ps")
                kp = ps.tile([128, C], F32, tag="ps")
                vp = ps.tile([128, C], F32, tag="ps")
                for kc in range(4):
                    for g, dst in enumerate((qp, kp, vp)):
                        nc.tensor.matmul(
                            dst,
                            mm(xts[:, kc, :]),
                            mm(qkv_wT[kc][:, g * C : (g + 1) * C]),
                            start=(kc == 0),
                            stop=(kc == 3),
                        )

                qks = chunk.tile([128, 2, C], op_dt, tag="qks")
                nc.scalar.copy(out=qks[:, 0, :], in_=qp)
                nc.vector.tensor_copy(out=qks[:, 1, :], in_=kp)
                vs = chunk.tile([128, C], op_dt, tag="vs")
                nc.scalar.copy(out=vs, in_=vp)

                # squared q,k for the sum-of-squares accumulators
                sq = chunk.tile([128, 2, C], op_dt, tag="sq")
                nc.gpsimd.tensor_mul(out=sq, in0=qks, in1=qks)

                # attn Gram accumulation (pair-packed): two independent
                # accumulation groups per bank (partitions 0-63 and 64-127),
                # each with exactly one start and one stop matmul.
                # (kept plain fp32 in f32r mode: free dim 64 gets no f32r
                # speedup anyway, and this matmul is precision-sensitive)
                for h in range(H):
                    r = h % 2
                    nc.tensor.matmul(
                        acc[r * 64 : r * 64 + 64, h // 2, :],
                        qks[:, 0, ts(h, D)],
                        qks[:, 1, ts(h, D)],
                        start=(ci == 0 and h < 2),
                        stop=(ci == NCHUNK - 1 and h >= H - 2),
                    )
                # sum-of-squares via ones-vector matmul
                nc.tensor.matmul(
                    ssq_q,
                    mm(ones_col),
                    mm(sq[:, 0, :]),
                    start=(ci == 0),
                    stop=(ci == NCHUNK - 1),
                )
                nc.tensor.matmul(
                    ssq_k,
                    mm(ones_col),
                    mm(sq[:, 1, :]),
                    start=(ci == 0),
                    stop=(ci == NCHUNK - 1),
                )

                # transpose v chunk to channel-major vT tiles
                vtps = ps.tile([128, 4, 128], op_dt, tag="ps")
                for g in range(4):
                    nc.tensor.transpose(vtps[:, g, :], vs[:, ts(g, 128)], ident)
                for g in range(4):
                    nc.vector.tensor_copy(
                        out=vT[g][:, ts(ci, 128)], in_=vtps[:, g, :]
                    )

            # ---------------- Phase B: softmax + out + proj ----------------
            # Everything pair-packed: [128 partitions, 4 pair slots, 64].
            attn = small.tile([128, 4, D], F32, tag="attn")
            nc.scalar.copy(out=attn, in_=acc)
            ssq = small.tile([1, 2, C], F32, tag="ssq")
            nc.vector.tensor_copy(out=ssq[:, 0, :], in_=ssq_q)
            nc.vector.tensor_copy(out=ssq[:, 1, :], in_=ssq_k)

            # inv norm = 1/max(sqrt(ssq), eps); fold temperature into q side
            nrm = small.tile([1, 2, H, D], F32, tag="nrm")
            nc.scalar.sqrt(out=nrm, in_=ssq.rearrange("p t (h d) -> p t h d", h=H))
            nc.vector.tensor_scalar_max(nrm, nrm, EPS)
            nc.vector.reciprocal(out=nrm, in_=nrm)
            temp_bc = bass.AP(
                tensor=temp_sb.tensor,
                offset=temp_sb.offset,
                ap=[list(temp_sb.ap[0]), [1, H], [0, D]],
            )
            nc.vector.tensor_tensor(
                out=nrm[:, 0], in0=nrm[:, 0], in1=temp_bc, op=mybir.AluOpType.mult
            )

            # alpha[p, j] = inv_q[ch] * temp for channel ch=(2j + p//64)*64 + p%64
            # via 4 tiny PE transposes of [1,128] slices -> [128,1] columns
            alpha_ps = ps.tile([128, 4], F32, tag="ps")
            for j in range(4):
                nc.tensor.transpose(
                    alpha_ps[:, j : j + 1],
                    nrm[0:1, 0].rearrange("p h d -> p (h d)")[:, ts(j, 128)],
                    id1_f32,
                )
            alpha = small.tile([128, 4], F32, tag="alpha")
            nc.vector.tensor_copy(out=alpha, in_=alpha_ps)

            # inv_k broadcast, pair-packed: partitions 0-63 get even heads,
            # 64-127 get odd heads (two K=1 ones-matmuls)
            ikb_ps = ps.tile([128, 4, D], F32, tag="ps")
            nrm_k = nrm[:, 1]  # [1, H, D]
            nc.tensor.matmul(
                ikb_ps[0:64], ones_f32[:, 0:64], nrm_k[:, 0::2, :],
                start=True, stop=True,
            )
            nc.tensor.matmul(
                ikb_ps[64:128], ones_f32[:, 0:64], nrm_k[:, 1::2, :],
                start=True, stop=True,
            )
            ikb = small.tile([128, 4, D], F32, tag="ikb")
            nc.vector.tensor_copy(out=ikb, in_=ikb_ps)

            # z = gram * inv_k (free axis) * alpha (per partition+slot)
            nc.vector.tensor_mul(out=attn, in0=attn, in1=ikb)
            alpha_bc = bass.AP(
                tensor=alpha.tensor,
                offset=alpha.offset,
                ap=[list(alpha.ap[0]), list(alpha.ap[1]), [0, D]],
            )
            nc.vector.tensor_tensor(
                out=attn, in0=attn, in1=alpha_bc, op=mybir.AluOpType.mult
            )

            # softmax over the last axis (per head)
            mx = small.tile([128, 4], F32, tag="mx")
            nc.vector.tensor_reduce(
                out=mx, in_=attn, axis=mybir.AxisListType.X,
                op=mybir.AluOpType.max, negate=True,
            )
            mx_bc = bass.AP(
                tensor=mx.tensor, offset=mx.offset,
                ap=[list(mx.ap[0]), list(mx.ap[1]), [0, D]],
            )
            nc.vector.tensor_tensor(
                out=attn, in0=attn, in1=mx_bc, op=mybir.AluOpType.add
            )
            ex = small.tile([128, 4, D], F32, tag="ex")
            nc.scalar.activation(
                out=ex, in_=attn, func=mybir.ActivationFunctionType.Exp
            )
            rs = small.tile([128, 4], F32, tag="rs")
            nc.vector.tensor_reduce(
                out=rs, in_=ex, axis=mybir.AxisListType.X, op=mybir.AluOpType.add
            )
            nc.vector.reciprocal(out=rs, in_=rs)
            probs = small.tile([128, 4, D], op_dt, tag="probs")
            rs_bc = bass.AP(
                tensor=rs.tensor, offset=rs.offset,
                ap=[list(rs.ap[0]), list(rs.ap[1]), [0, D]],
            )
            nc.vector.tensor_tensor(
                out=probs, in0=ex, in1=rs_bc, op=mybir.AluOpType.mult
            )

            # transpose probs (per head) -> attnT, same pair-packed layout
            atp = ps.tile([128, 4, D], op_dt, tag="ps")
            for h in range(H):
                r = h % 2
                sl = slice(r * 64, r * 64 + 64)
                nc.tensor.transpose(
                    atp[sl, h // 2, :],
                    probs[sl, h // 2, :],
                    ident[sl, sl],
                )
            attnT = small.tile([128, 4, D], op_dt, tag="attnT")
            nc.vector.tensor_copy(out=attnT, in_=atp)

            # out = attn @ v (channel-major), then proj back to token-major
            for nj in range(N // 512):
                outT = outp.tile([128, 4, 512], op_dt, tag="outT")
                for g in range(4):
                    ops = ps.tile([128, 512], F32, tag="ps")
                    for r in range(2):
                        sl = slice(r * 64, r * 64 + 64)
                        nc.tensor.matmul(
                            ops[sl, :],
                            mm(attnT[sl, g, :]),
                            mm(vT[g][sl, ts(nj, 512)]),
                            start=True,
                            stop=True,
                        )
                    nc.scalar.copy(out=outT[:, g, :], in_=ops)
                for t4 in range(4):
                    ypt = ps.tile([128, 512], F32, tag="ps")
                    for kc in range(4):
                        nc.tensor.matmul(
                            ypt,
                            mm(outT[:, kc, ts(t4, 128)]),
                            mm(proj_wT[kc]),
                            start=(kc == 0),
                            stop=(kc == 3),
                        )
                    ysb = yp.tile([128, C], F32, tag="ysb")
                    nc.vector.tensor_add(out=ysb, in0=ypt, in1=bias_bc)
                    nc.sync.dma_start(
                        out=out[b, nj * 512 + t4 * 128 : nj * 512 + (t4 + 1) * 128, :],
                        in_=ysb,
                    )

        accp.release()
        ps.release()
        yp.release()
        outp.release()
        small.release()
        vtp.release()
        chunk.release()
        xin.release()
        wstage.release()
        consts.release()

    legalize_waits(nc)
    return nc


def build_trivial_bass():
    """Minimal kernel used by the benchmark harness to measure the
    per-dispatch floor (axon round trip + runtime overhead)."""
    nc = bass.Bass(trn_type="TRN2")
    inp = nc.dram_tensor("inp", [128, 512], F32, kind="ExternalInput")
    outp = nc.dram_tensor("outp", [128, 512], F32, kind="ExternalOutput")
    with TileContext(nc) as tc:
        with tc.tile_pool(name="p", bufs=1) as pool:
            s = pool.tile([128, 512], F32)
            nc.sync.dma_start(out=s, in_=inp[:, :])
            nc.sync.dma_start(out=outp[:, :], in_=s)
    legalize_waits(nc)
    return nc


_NC_CACHE = {}


def kernel(x, qkv_w, temperature, proj_w, proj_b, _want_trace=False, _trace_kwargs=None):
    x = np.ascontiguousarray(x, dtype=np.float32)
    key = MM_MODE
    if key not in _NC_CACHE:
        _NC_CACHE[key] = build_bass()
    nc = _NC_CACHE[key]

    temp_flat = np.ascontiguousarray(np.asarray(temperature, np.float32).reshape(H))
    in_maps = []
    for i in range(NCORES):
        in_maps.append(
            {
                "x": np.ascontiguousarray(x[i * BL : (i + 1) * BL]),
                "qkv_w": np.ascontiguousarray(qkv_w, np.float32),
                "temperature": temp_flat,
                "proj_w": np.ascontiguousarray(proj_w, np.float32),
                "proj_b": np.ascontiguousarray(proj_b, np.float32),
            }
        )
    res = run_bass_kernel_spmd(
        nc,
        in_maps,
        core_ids=list(range(NCORES)),
        trace=_want_trace,
        **(_trace_kwargs or {}),
    )
    y = np.concatenate([res.results[i]["out"] for i in range(NCORES)], axis=0)
    if _want_trace:
        return y, res
    return y


# revision 36
# speedup vs baseline: 9.3022x; 9.3022x over previous
"""Trainium2 Bass kernel for nn_CABlock (channel attention / XCA block).

Reference computation (per batch b):
  qkv = x @ qkv_w.T                      # [N, 3C], token-major
  q,k,v per head: [d=64, N] channel-major after reshape/transpose
  q,k l2-normalized over N; attn = softmax((q @ k.T) * temperature, axis=-1)
  out = attn @ v  -> [N, C];  y = out @ proj_w.T + proj_b

Key algebraic restructure: l2norm commutes with the bilinear form, so
  attn_logits = diag(inv_q) @ (q_raw @ k_raw.T) @ diag(inv_k) * temp
with inv_q[c] = 1/max(||q[c,:]||,eps). We accumulate q^T k Gram blocks and
per-channel sums of squares (via a ones-vector matmul over squared values)
in PSUM across all token chunks, then apply the tiny [64x64]-per-head
softmax at the end. This avoids ever materializing normalized q/k.

Sharding: data-parallel over batch B=16 across 8 cores (2 batches/core).
No collectives needed.
"""

import os
import sys

import numpy as np

for _p in ("/opt/trn_rl_repo", "/root/.axon_site/_ro/trn_rl_repo"):
    if os.path.isdir(_p) and _p not in sys.path:
        sys.path.insert(0, _p)

import concourse.bass as bass  # noqa: E402
from concourse import mybir  # noqa: E402
from concourse.bass import ts  # noqa: E402
from concourse.bass_utils import run_bass_kernel_spmd  # noqa: E402
from concourse.masks import make_identity  # noqa: E402
from concourse.tile import TileContext  # noqa: E402

B, N, C = 16, 4096, 512
H, D = 8, 64
C3 = 3 * C
NCORES = 8
BL = B // NCORES  # batches per core
EPS = 1e-12
NCHUNK = N // 128  # 32 token chunks per batch
F32 = mybir.dt.float32

# Matmul operand mode: "bf16" | "fp32" | "f32r" (set BASS_MM_MODE to override)
MM_MODE = os.environ.get("BASS_MM_MODE", "bf16")


def legalize_waits(nc):
    """Walrus in this environment rejects instructions carrying more than one
    semaphore wait ("Too many sync wait commands"), and rejects sem-ge waits
    on Drain instructions entirely. Tile emits both. Hoist the offending
    waits onto standalone EventSemaphore instructions inserted immediately
    before the instruction on the same engine queue — semantically identical
    (the engine executes the waits, then the instruction)."""
    n_new = 0
    for bb in nc.main_func.blocks:
        il = bb.instructions
        new_list = []
        for ins in il:
            si = ins.sync_info
            waits = list(si.on_wait) if si is not None and si.on_wait else []
            if waits:
                tname = type(ins).__name__
                no_wait_slots = tname in ("InstDrain", "InstDmaTransposeAnt") or (
                    getattr(ins, "opcode", "") in ("Drain", "DmaTransposeAnt")
                )
                keep_budget = 0 if no_wait_slots else 1
                if len(waits) > keep_budget:
                    hoist, keep = waits[:-keep_budget] if keep_budget else waits, (
                        waits[-keep_budget:] if keep_budget else []
                    )
                    for w in hoist:
                        ev = mybir.InstEventSemaphore(
                            name=f"{ins.name}-hoistw{n_new}",
                            ins=[],
                            outs=[],
                            engine=ins.engine,
                            sync_info=mybir.SyncInfo(on_wait=[w], on_update=[]),
                        )
                        new_list.append(ev)
                        n_new += 1
                    ins.sync_info = mybir.SyncInfo(
                        on_wait=keep, on_update=list(si.on_update or [])
                    )
            new_list.append(ins)
        il.clear()
        il.extend(new_list)
    return n_new


def build_bass():
    mode = MM_MODE
    op_dt = mybir.dt.bfloat16 if mode == "bf16" else F32

    def mm(ap):
        """Cast an operand AP at a matmul call site for the big matmuls."""
        if mode == "f32r":
            return ap.bitcast(mybir.dt.float32r)
        return ap

    nc = bass.Bass(trn_type="TRN2")
    x = nc.dram_tensor("x", [BL, N, C], F32, kind="ExternalInput")
    # weights arrive pre-transposed ([cin, cout]) and pre-converted to the
    # matmul dtype by the host wrapper
    qkv_wt = nc.dram_tensor("qkv_wt", [C, C3], op_dt, kind="ExternalInput")
    temp = nc.dram_tensor("temperature", [H], F32, kind="ExternalInput")
    proj_wt = nc.dram_tensor("proj_wt", [C, C], op_dt, kind="ExternalInput")
    proj_b = nc.dram_tensor("proj_b", [C], F32, kind="ExternalInput")
    out = nc.dram_tensor("out", [BL, N, C], F32, kind="ExternalOutput")

    with TileContext(nc) as tc:
        consts = tc.alloc_tile_pool(name="consts", bufs=1)
        xin = tc.alloc_tile_pool(name="xin", bufs=6)
        chunk = tc.alloc_tile_pool(name="chunk", bufs=3)
        # fp32 modes double the vT footprint; drop cross-batch double-buffering
        vtp = tc.alloc_tile_pool(name="vtp", bufs=8 if op_dt != F32 else 4)
        small = tc.alloc_tile_pool(name="small", bufs=2)
        outp = tc.alloc_tile_pool(name="outp", bufs=2)
        yp = tc.alloc_tile_pool(name="yp", bufs=3)
        ps = tc.alloc_tile_pool(name="ps", bufs=5, space="PSUM")
        accp = tc.alloc_tile_pool(name="accp", bufs=1, space="PSUM")

        # ---- constants ----
        ident = consts.tile([128, 128], op_dt)
        make_identity(nc, ident)
        ones_col = consts.tile([128, 1], op_dt)
        nc.vector.memset(ones_col, 1.0)
        ones_f32 = consts.tile([1, 128], F32)
        nc.vector.memset(ones_f32, 1.0)
        id1_f32 = consts.tile([1, 1], F32)
        nc.vector.memset(id1_f32, 1.0)

        # temperature: [1, H] row (broadcast along D later via 0-step APs)
        temp_sb = consts.tile([1, H], F32)
        nc.sync.dma_start(out=temp_sb, in_=temp[:])

        # proj bias: load [1, C] then broadcast to all partitions via a
        # K=1 ones-matmul on the PE (out[p, c] = 1 * bias[c])
        bias_row = consts.tile([1, C], F32)
        nc.sync.dma_start(out=bias_row, in_=proj_b[:])
        bias_ps = ps.tile([128, C], F32, tag="ps")
        nc.tensor.matmul(bias_ps, ones_f32, bias_row, start=True, stop=True)
        bias_bc = consts.tile([128, C], F32)
        nc.vector.tensor_copy(out=bias_bc, in_=bias_ps)

        # ---- weights: already [cin, cout] in matmul dtype; plain DMA loads ----
        qkv_wT = [consts.tile([128, C3], op_dt, tag=f"qkvwT{i}", name=f"qkvwT{i}") for i in range(4)]
        proj_wT = [consts.tile([128, C], op_dt, tag=f"projwT{i}", name=f"projwT{i}") for i in range(4)]
        for kc in range(4):
            nc.sync.dma_start(out=qkv_wT[kc], in_=qkv_wt[ts(kc, 128), :])
            nc.sync.dma_start(out=proj_wT[kc], in_=proj_wt[ts(kc, 128), :])

        for b in range(BL):
            # Persistent per-batch PSUM accumulators. Heads are "pair-packed":
            # head h lives at partitions (h%2)*64..(h%2)*64+63.
            # Gram pair-matmuls write [128, 128] blocks per head pair j; the
            # useful data is the diagonal sub-blocks:
            #   acc2[r*64:+64, j, r*64:+64] = sum_n q_h^T k_h  (h = 2j + r)
            acc2 = accp.tile([128, 4, 128], F32, tag="acc")
            ssq_q = accp.tile([1, C], F32, tag="ssq_q")
            ssq_k = accp.tile([1, C], F32, tag="ssq_k")
            # x and v in channel-major layout (tiles per 128-channel group)
            xT = [
                vtp.tile([128, N], op_dt, tag="xt_cm", name=f"xtcm{g}", bufs=4)
                for g in range(4)
            ]
            vT = [vtp.tile([128, N], op_dt, tag="vt", name=f"vt{g}") for g in range(4)]

            # ---------------- Phase A: stream token chunks ----------------
            for ci in range(NCHUNK):
                xt = xin.tile([128, C], F32, tag="xt")
                nc.sync.dma_start(out=xt, in_=x[b, ts(ci, 128), :])
                xb = chunk.tile([128, C], op_dt, tag="xb", bufs=6)
                nc.gpsimd.tensor_copy(out=xb, in_=xt)
                # transpose x chunk into the channel-major batch buffer
                xtp = ps.tile([128, 4, 128], op_dt, tag="ps")
                for g in range(4):
                    nc.tensor.transpose(xtp[:, g, :], xb[:, ts(g, 128)], ident)
                for g in range(4):
                    nc.vector.tensor_copy(out=xT[g][:, ts(ci, 128)], in_=xtp[:, g, :])

                # q,k projection, token-major: stationary = xT chunk
                qp = ps.tile([128, C], F32, tag="ps")
                kp = ps.tile([128, C], F32, tag="ps")
                for kc in range(4):
                    for g, dst in enumerate((qp, kp)):
                        nc.tensor.matmul(
                            dst,
                            mm(xT[kc][:, ts(ci, 128)]),
                            mm(qkv_wT[kc][:, g * C : (g + 1) * C]),
                            start=(kc == 0),
                            stop=(kc == 3),
                        )

                qks = chunk.tile([128, 2, C], op_dt, tag="qks")
                nc.scalar.copy(out=qks[:, 0, :], in_=qp)
                nc.vector.tensor_copy(out=qks[:, 1, :], in_=kp)

                # squared q,k for the sum-of-squares accumulators
                sq = chunk.tile([128, 2, C], op_dt, tag="sq")
                nc.gpsimd.tensor_mul(out=sq, in0=qks, in1=qks)

                # attn Gram accumulation, one [128,128] matmul per head pair
                # (off-diagonal blocks are computed but unused). Two
                # independent accumulation groups per bank (partitions 0-63
                # and 64-127), each with exactly one start and stop.
                for j in range(4):
                    nc.tensor.matmul(
                        acc2[:, j, :],
                        qks[:, 0, ts(j, 2 * D)],
                        qks[:, 1, ts(j, 2 * D)],
                        start=(ci == 0 and j == 0),
                        stop=(ci == NCHUNK - 1 and j == 3),
                    )
                # sum-of-squares via ones-vector matmul
                nc.tensor.matmul(
                    ssq_q,
                    mm(ones_col),
                    mm(sq[:, 0, :]),
                    start=(ci == 0),
                    stop=(ci == NCHUNK - 1),
                )
                nc.tensor.matmul(
                    ssq_k,
                    mm(ones_col),
                    mm(sq[:, 1, :]),
                    start=(ci == 0),
                    stop=(ci == NCHUNK - 1),
                )

                # v projection for each completed 512-token group,
                # channel-major directly: stationary = v weight block
                # [cin, cout128], moving = xT 512-token slices
                if ci % 4 == 3:
                    nj = ci // 4
                    for mc in range(4):
                        vps = ps.tile([128, 512], F32, tag="ps")
                        for kc in range(4):
                            nc.tensor.matmul(
                                vps,
                                mm(qkv_wT[kc][:, 2 * C + mc * 128 : 2 * C + (mc + 1) * 128]),
                                mm(xT[kc][:, ts(nj, 512)]),
                                start=(kc == 0),
                                stop=(kc == 3),
                            )
                        nc.scalar.copy(out=vT[mc][:, ts(nj, 512)], in_=vps)

            # ---------------- Phase B: softmax + out + proj ----------------
            # Everything pair-packed: [128 partitions, 4 pair slots, 64].
            attn = small.tile([128, 4, D], F32, tag="attn")
            nc.scalar.copy(out=attn[0:64], in_=acc2[0:64, :, 0:64])
            nc.scalar.copy(out=attn[64:128], in_=acc2[64:128, :, 64:128])
            ssq = small.tile([1, 2, C], F32, tag="ssq")
            nc.vector.tensor_copy(out=ssq[:, 0, :], in_=ssq_q)
            nc.vector.tensor_copy(out=ssq[:, 1, :], in_=ssq_k)

            # inv norm = 1/max(sqrt(ssq), eps); fold temperature into q side
            nrm = small.tile([1, 2, H, D], F32, tag="nrm")
            nc.scalar.sqrt(out=nrm, in_=ssq.rearrange("p t (h d) -> p t h d", h=H))
            nc.vector.tensor_scalar_max(nrm, nrm, EPS)
            nc.vector.reciprocal(out=nrm, in_=nrm)
            temp_bc = bass.AP(
                tensor=temp_sb.tensor,
                offset=temp_sb.offset,
                ap=[list(temp_sb.ap[0]), [1, H], [0, D]],
            )
            nc.vector.tensor_tensor(
                out=nrm[:, 0], in0=nrm[:, 0], in1=temp_bc, op=mybir.AluOpType.mult
            )

            # alpha[p, j] = inv_q[ch] * temp for channel ch=(2j + p//64)*64 + p%64
            # via 4 tiny PE transposes of [1,128] slices -> [128,1] columns
            alpha_ps = ps.tile([128, 4], F32, tag="ps")
            for j in range(4):
                nc.tensor.transpose(
                    alpha_ps[:, j : j + 1],
                    nrm[0:1, 0].rearrange("p h d -> p (h d)")[:, ts(j, 128)],
                    id1_f32,
                )
            alpha = small.tile([128, 4], F32, tag="alpha")
            nc.vector.tensor_copy(out=alpha, in_=alpha_ps)

            # inv_k broadcast, pair-packed: partitions 0-63 get even heads,
            # 64-127 get odd heads (two K=1 ones-matmuls)
            ikb_ps = ps.tile([128, 4, D], F32, tag="ps")
            nrm_k = nrm[:, 1]  # [1, H, D]
            nc.tensor.matmul(
                ikb_ps[0:64], ones_f32[:, 0:64], nrm_k[:, 0::2, :],
                start=True, stop=True,
            )
            nc.tensor.matmul(
                ikb_ps[64:128], ones_f32[:, 0:64], nrm_k[:, 1::2, :],
                start=True, stop=True,
            )
            ikb = small.tile([128, 4, D], F32, tag="ikb")
            nc.vector.tensor_copy(out=ikb, in_=ikb_ps)

            # z = gram * inv_k (free axis) * alpha (per partition+slot)
            nc.vector.tensor_mul(out=attn, in0=attn, in1=ikb)
            alpha_bc = bass.AP(
                tensor=alpha.tensor,
                offset=alpha.offset,
                ap=[list(alpha.ap[0]), list(alpha.ap[1]), [0, D]],
            )
            nc.vector.tensor_tensor(
                out=attn, in0=attn, in1=alpha_bc, op=mybir.AluOpType.mult
            )

            # softmax over the last axis (per head)
            mx = small.tile([128, 4], F32, tag="mx")
            nc.vector.tensor_reduce(
                out=mx, in_=attn, axis=mybir.AxisListType.X,
                op=mybir.AluOpType.max, negate=True,
            )
            mx_bc = bass.AP(
                tensor=mx.tensor, offset=mx.offset,
                ap=[list(mx.ap[0]), list(mx.ap[1]), [0, D]],
            )
            nc.vector.tensor_tensor(
                out=attn, in0=attn, in1=mx_bc, op=mybir.AluOpType.add
            )
            ex = small.tile([128, 4, D], F32, tag="ex")
            nc.scalar.activation(
                out=ex, in_=attn, func=mybir.ActivationFunctionType.Exp
            )
            rs = small.tile([128, 4], F32, tag="rs")
            nc.vector.tensor_reduce(
                out=rs, in_=ex, axis=mybir.AxisListType.X, op=mybir.AluOpType.add
            )
            nc.vector.reciprocal(out=rs, in_=rs)
            probs = small.tile([128, 4, D], op_dt, tag="probs")
            rs_bc = bass.AP(
                tensor=rs.tensor, offset=rs.offset,
                ap=[list(rs.ap[0]), list(rs.ap[1]), [0, D]],
            )
            nc.vector.tensor_tensor(
                out=probs, in0=ex, in1=rs_bc, op=mybir.AluOpType.mult
            )

            # transpose probs (per head) -> attnT, same pair-packed layout
            atp = ps.tile([128, 4, D], op_dt, tag="ps")
            for h in range(H):
                r = h % 2
                sl = slice(r * 64, r * 64 + 64)
                nc.tensor.transpose(
                    atp[sl, h // 2, :],
                    probs[sl, h // 2, :],
                    ident[sl, sl],
                )
            attnT = small.tile([128, 4, D], op_dt, tag="attnT")
            nc.vector.tensor_copy(out=attnT, in_=atp)

            # out = attn @ v (channel-major), then proj back to token-major
            for nj in range(N // 512):
                outT = outp.tile([128, 4, 512], op_dt, tag="outT")
                for g in range(4):
                    ops = ps.tile([128, 512], F32, tag="ps")
                    for r in range(2):
                        sl = slice(r * 64, r * 64 + 64)
                        nc.tensor.matmul(
                            ops[sl, :],
                            mm(attnT[sl, g, :]),
                            mm(vT[g][sl, ts(nj, 512)]),
                            start=True,
                            stop=True,
                        )
                    nc.scalar.copy(out=outT[:, g, :], in_=ops)
                for t4 in range(4):
                    ypt = ps.tile([128, 512], F32, tag="ps")
                    for kc in range(4):
                        nc.tensor.matmul(
                            ypt,
                            mm(outT[:, kc, ts(t4, 128)]),
                            mm(proj_wT[kc]),
                            start=(kc == 0),
                            stop=(kc == 3),
                        )
                    ysb = yp.tile([128, C], F32, tag="ysb")
                    nc.vector.tensor_add(out=ysb, in0=ypt, in1=bias_bc)
                    nc.sync.dma_start(
                        out=out[b, nj * 512 + t4 * 128 : nj * 512 + (t4 + 1) * 128, :],
                        in_=ysb,
                    )

        accp.release()
        ps.release()
        yp.release()
        outp.release()
        small.release()
        vtp.release()
        chunk.release()
        xin.release()
        consts.release()

    legalize_waits(nc)
    return nc


def build_trivial_bass():
    """Minimal kernel used by the benchmark harness to measure the
    per-dispatch floor (axon round trip + runtime overhead)."""
    nc = bass.Bass(trn_type="TRN2")
    inp = nc.dram_tensor("inp", [128, 512], F32, kind="ExternalInput")
    outp = nc.dram_tensor("outp", [128, 512], F32, kind="ExternalOutput")
    with TileContext(nc) as tc:
        with tc.tile_pool(name="p", bufs=1) as pool:
            s = pool.tile([128, 512], F32)
            nc.sync.dma_start(out=s, in_=inp[:, :])
            nc.sync.dma_start(out=outp[:, :], in_=s)
    legalize_waits(nc)
    return nc


_NC_CACHE = {}


def kernel(x, qkv_w, temperature, proj_w, proj_b, _want_trace=False, _trace_kwargs=None):
    x = np.ascontiguousarray(x, dtype=np.float32)
    key = MM_MODE
    if key not in _NC_CACHE:
        _NC_CACHE[key] = build_bass()
    nc = _NC_CACHE[key]

    temp_flat = np.ascontiguousarray(np.asarray(temperature, np.float32).reshape(H))
    if MM_MODE == "bf16":
        import ml_dtypes

        w_dt = ml_dtypes.bfloat16
    else:
        w_dt = np.float32
    qkv_wt = np.ascontiguousarray(np.asarray(qkv_w, np.float32).T.astype(w_dt))
    proj_wt = np.ascontiguousarray(np.asarray(proj_w, np.float32).T.astype(w_dt))
    in_maps = []
    for i in range(NCORES):
        in_maps.append(
            {
                "x": np.ascontiguousarray(x[i * BL : (i + 1) * BL]),
                "qkv_wt": qkv_wt,
                "temperature": temp_flat,
                "proj_wt": proj_wt,
                "proj_b": np.ascontiguousarray(proj_b, np.float32),
            }
        )
    res = run_bass_kernel_spmd(
        nc,
        in_maps,
        core_ids=list(range(NCORES)),
        trace=_want_trace,
        **(_trace_kwargs or {}),
    )
    y = np.concatenate([res.results[i]["out"] for i in range(NCORES)], axis=0)
    if _want_trace:
        return y, res
    return y


# revision 39
# speedup vs baseline: 9.8519x; 1.0591x over previous
"""Trainium2 Bass kernel for nn_CABlock (channel attention / XCA block).

Reference computation (per batch b):
  qkv = x @ qkv_w.T                      # [N, 3C], token-major
  q,k,v per head: [d=64, N] channel-major after reshape/transpose
  q,k l2-normalized over N; attn = softmax((q @ k.T) * temperature, axis=-1)
  out = attn @ v  -> [N, C];  y = out @ proj_w.T + proj_b

Key algebraic restructure: l2norm commutes with the bilinear form, so
  attn_logits = diag(inv_q) @ (q_raw @ k_raw.T) @ diag(inv_k) * temp
with inv_q[c] = 1/max(||q[c,:]||,eps). We accumulate q^T k Gram blocks and
per-channel sums of squares (via a ones-vector matmul over squared values)
in PSUM across all token chunks, then apply the tiny [64x64]-per-head
softmax at the end. This avoids ever materializing normalized q/k.

Sharding: data-parallel over batch B=16 across 8 cores (2 batches/core).
No collectives needed.
"""

import os
import sys

import numpy as np

for _p in ("/opt/trn_rl_repo", "/root/.axon_site/_ro/trn_rl_repo"):
    if os.path.isdir(_p) and _p not in sys.path:
        sys.path.insert(0, _p)

import concourse.bass as bass  # noqa: E402
from concourse import mybir  # noqa: E402
from concourse.bass import ts  # noqa: E402
from concourse.bass_utils import run_bass_kernel_spmd  # noqa: E402
from concourse.masks import make_identity  # noqa: E402
from concourse.tile import TileContext  # noqa: E402

B, N, C = 16, 4096, 512
H, D = 8, 64
C3 = 3 * C
NCORES = 8
BL = B // NCORES  # batches per core
EPS = 1e-12
NCHUNK = N // 128  # 32 token chunks per batch
F32 = mybir.dt.float32

# Matmul operand mode: "bf16" | "fp32" | "f32r" (set BASS_MM_MODE to override)
MM_MODE = os.environ.get("BASS_MM_MODE", "bf16")


def legalize_waits(nc):
    """Walrus in this environment rejects instructions carrying more than one
    semaphore wait ("Too many sync wait commands"), and rejects sem-ge waits
    on Drain instructions entirely. Tile emits both. Hoist the offending
    waits onto standalone EventSemaphore instructions inserted immediately
    before the instruction on the same engine queue — semantically identical
    (the engine executes the waits, then the instruction)."""
    n_new = 0
    for bb in nc.main_func.blocks:
        il = bb.instructions
        new_list = []
        for ins in il:
            si = ins.sync_info
            waits = list(si.on_wait) if si is not None and si.on_wait else []
            if waits:
                tname = type(ins).__name__
                no_wait_slots = tname in ("InstDrain", "InstDmaTransposeAnt") or (
                    getattr(ins, "opcode", "") in ("Drain", "DmaTransposeAnt")
                )
                keep_budget = 0 if no_wait_slots else 1
                if len(waits) > keep_budget:
                    hoist, keep = waits[:-keep_budget] if keep_budget else waits, (
                        waits[-keep_budget:] if keep_budget else []
                    )
                    for w in hoist:
                        ev = mybir.InstEventSemaphore(
                            name=f"{ins.name}-hoistw{n_new}",
                            ins=[],
                            outs=[],
                            engine=ins.engine,
                            sync_info=mybir.SyncInfo(on_wait=[w], on_update=[]),
                        )
                        new_list.append(ev)
                        n_new += 1
                    ins.sync_info = mybir.SyncInfo(
                        on_wait=keep, on_update=list(si.on_update or [])
                    )
            new_list.append(ins)
        il.clear()
        il.extend(new_list)
    return n_new


def build_bass():
    mode = MM_MODE
    op_dt = mybir.dt.bfloat16 if mode == "bf16" else F32

    def mm(ap):
        """Cast an operand AP at a matmul call site for the big matmuls."""
        if mode == "f32r":
            return ap.bitcast(mybir.dt.float32r)
        return ap

    nc = bass.Bass(trn_type="TRN2")
    x = nc.dram_tensor("x", [BL, N, C], F32, kind="ExternalInput")
    # weights arrive pre-transposed ([cin, cout]) and pre-converted to the
    # matmul dtype by the host wrapper
    qkv_wt = nc.dram_tensor("qkv_wt", [C, C3], op_dt, kind="ExternalInput")
    temp = nc.dram_tensor("temperature", [H], F32, kind="ExternalInput")
    proj_wt = nc.dram_tensor("proj_wt", [C, C], op_dt, kind="ExternalInput")
    proj_b = nc.dram_tensor("proj_b", [C], F32, kind="ExternalInput")
    out = nc.dram_tensor("out", [BL, N, C], F32, kind="ExternalOutput")

    with TileContext(nc) as tc:
        consts = tc.alloc_tile_pool(name="consts", bufs=1)
        xin = tc.alloc_tile_pool(name="xin", bufs=6)
        chunk = tc.alloc_tile_pool(name="chunk", bufs=3)
        # fp32 modes double the vT footprint; drop cross-batch double-buffering
        vtp = tc.alloc_tile_pool(name="vtp", bufs=8 if op_dt != F32 else 4)
        small = tc.alloc_tile_pool(name="small", bufs=2)
        outp = tc.alloc_tile_pool(name="outp", bufs=2)
        yp = tc.alloc_tile_pool(name="yp", bufs=3)
        ps = tc.alloc_tile_pool(name="ps", bufs=5, space="PSUM")
        accp = tc.alloc_tile_pool(name="accp", bufs=1, space="PSUM")

        # ---- constants ----
        ident = consts.tile([128, 128], op_dt)
        make_identity(nc, ident)
        ones_col = consts.tile([128, 1], op_dt)
        nc.vector.memset(ones_col, 1.0)
        ones_f32 = consts.tile([1, 128], F32)
        nc.vector.memset(ones_f32, 1.0)
        id1_f32 = consts.tile([1, 1], F32)
        nc.vector.memset(id1_f32, 1.0)

        # temperature: [1, H] row (broadcast along D later via 0-step APs)
        temp_sb = consts.tile([1, H], F32)
        nc.sync.dma_start(out=temp_sb, in_=temp[:])

        # proj bias: load [1, C] then broadcast to all partitions via a
        # K=1 ones-matmul on the PE (out[p, c] = 1 * bias[c])
        bias_row = consts.tile([1, C], F32)
        nc.sync.dma_start(out=bias_row, in_=proj_b[:])
        bias_ps = ps.tile([128, C], F32, tag="ps")
        nc.tensor.matmul(bias_ps, ones_f32, bias_row, start=True, stop=True)
        bias_bc = consts.tile([128, C], F32)
        nc.vector.tensor_copy(out=bias_bc, in_=bias_ps)

        # ---- weights: already [cin, cout] in matmul dtype; plain DMA loads ----
        qkv_wT = [consts.tile([128, C3], op_dt, tag=f"qkvwT{i}", name=f"qkvwT{i}") for i in range(4)]
        proj_wT = [consts.tile([128, C], op_dt, tag=f"projwT{i}", name=f"projwT{i}") for i in range(4)]
        for kc in range(4):
            nc.sync.dma_start(out=qkv_wT[kc], in_=qkv_wt[ts(kc, 128), :])
            nc.sync.dma_start(out=proj_wT[kc], in_=proj_wt[ts(kc, 128), :])

        for b in range(BL):
            # Persistent per-batch PSUM accumulators. Heads are "pair-packed":
            # head h lives at partitions (h%2)*64..(h%2)*64+63.
            # Gram pair-matmuls write [128, 128] blocks per head pair j; the
            # useful data is the diagonal sub-blocks:
            #   acc2[r*64:+64, j, r*64:+64] = sum_n q_h^T k_h  (h = 2j + r)
            acc2 = accp.tile([128, 4, 128], F32, tag="acc")
            ssq_q = accp.tile([1, C], F32, tag="ssq_q")
            ssq_k = accp.tile([1, C], F32, tag="ssq_k")
            # x and v in channel-major layout (tiles per 128-channel group)
            xT = [
                vtp.tile([128, N], op_dt, tag="xt_cm", name=f"xtcm{g}", bufs=4)
                for g in range(4)
            ]
            vT = [vtp.tile([128, N], op_dt, tag="vt", name=f"vt{g}") for g in range(4)]

            # ---------------- Phase A: stream token chunks ----------------
            for ci in range(NCHUNK):
                xt = xin.tile([128, C], F32, tag="xt")
                nc.sync.dma_start(out=xt, in_=x[b, ts(ci, 128), :])
                xb = chunk.tile([128, C], op_dt, tag="xb", bufs=6)
                nc.gpsimd.tensor_copy(out=xb, in_=xt)
                # transpose x chunk into the channel-major batch buffer
                xtp = ps.tile([128, 4, 128], op_dt, tag="ps")
                for g in range(4):
                    nc.tensor.transpose(xtp[:, g, :], xb[:, ts(g, 128)], ident)
                for g in range(4):
                    nc.vector.tensor_copy(out=xT[g][:, ts(ci, 128)], in_=xtp[:, g, :])

                # q,k projection, token-major: stationary = xT chunk
                qp = ps.tile([128, C], F32, tag="ps")
                kp = ps.tile([128, C], F32, tag="ps")
                for kc in range(4):
                    for g, dst in enumerate((qp, kp)):
                        nc.tensor.matmul(
                            dst,
                            mm(xT[kc][:, ts(ci, 128)]),
                            mm(qkv_wT[kc][:, g * C : (g + 1) * C]),
                            start=(kc == 0),
                            stop=(kc == 3),
                        )

                qks = chunk.tile([128, 2, C], op_dt, tag="qks")
                nc.scalar.copy(out=qks[:, 0, :], in_=qp)
                nc.vector.tensor_copy(out=qks[:, 1, :], in_=kp)

                # squared q,k for the sum-of-squares accumulators
                sq = chunk.tile([128, 2, C], op_dt, tag="sq")
                nc.gpsimd.tensor_mul(out=sq, in0=qks, in1=qks)

                # attn Gram accumulation, one [128,128] matmul per head pair
                # (off-diagonal blocks are computed but unused). Two
                # independent accumulation groups per bank (partitions 0-63
                # and 64-127), each with exactly one start and stop.
                for j in range(4):
                    nc.tensor.matmul(
                        acc2[:, j, :],
                        qks[:, 0, ts(j, 2 * D)],
                        qks[:, 1, ts(j, 2 * D)],
                        start=(ci == 0 and j == 0),
                        stop=(ci == NCHUNK - 1 and j == 3),
                    )
                # sum-of-squares via ones-vector matmul
                nc.tensor.matmul(
                    ssq_q,
                    mm(ones_col),
                    mm(sq[:, 0, :]),
                    start=(ci == 0),
                    stop=(ci == NCHUNK - 1),
                )
                nc.tensor.matmul(
                    ssq_k,
                    mm(ones_col),
                    mm(sq[:, 1, :]),
                    start=(ci == 0),
                    stop=(ci == NCHUNK - 1),
                )

                # v projection, channel-major directly: stationary = v weight
                # block [cin, cout128], moving = xT 512-token slices. One
                # (nj, mc) sub-block per chunk to keep PSUM pool pressure flat;
                # group nj's inputs are ready once chunk 4*nj+3 is transposed.
                vjobs = []
                if ci >= 4:
                    vjobs.append((ci // 4 - 1, ci % 4))
                if ci == NCHUNK - 1:
                    vjobs += [(NCHUNK // 4 - 1, mc) for mc in range(4)]
                for nj, mc in vjobs:
                    vps = ps.tile([128, 512], F32, tag="ps")
                    for kc in range(4):
                        nc.tensor.matmul(
                            vps,
                            mm(qkv_wT[kc][:, 2 * C + mc * 128 : 2 * C + (mc + 1) * 128]),
                            mm(xT[kc][:, ts(nj, 512)]),
                            start=(kc == 0),
                            stop=(kc == 3),
                        )
                    nc.scalar.copy(out=vT[mc][:, ts(nj, 512)], in_=vps)

            # ---------------- Phase B: softmax + out + proj ----------------
            # Everything pair-packed: [128 partitions, 4 pair slots, 64].
            attn = small.tile([128, 4, D], F32, tag="attn")
            nc.scalar.copy(out=attn[0:64], in_=acc2[0:64, :, 0:64])
            nc.scalar.copy(out=attn[64:128], in_=acc2[64:128, :, 64:128])
            ssq = small.tile([1, 2, C], F32, tag="ssq")
            nc.vector.tensor_copy(out=ssq[:, 0, :], in_=ssq_q)
            nc.vector.tensor_copy(out=ssq[:, 1, :], in_=ssq_k)

            # inv norm = 1/max(sqrt(ssq), eps); fold temperature into q side
            nrm = small.tile([1, 2, H, D], F32, tag="nrm")
            nc.scalar.sqrt(out=nrm, in_=ssq.rearrange("p t (h d) -> p t h d", h=H))
            nc.vector.tensor_scalar_max(nrm, nrm, EPS)
            nc.vector.reciprocal(out=nrm, in_=nrm)
            temp_bc = bass.AP(
                tensor=temp_sb.tensor,
                offset=temp_sb.offset,
                ap=[list(temp_sb.ap[0]), [1, H], [0, D]],
            )
            nc.vector.tensor_tensor(
                out=nrm[:, 0], in0=nrm[:, 0], in1=temp_bc, op=mybir.AluOpType.mult
            )

            # alpha[p, j] = inv_q[ch] * temp for channel ch=(2j + p//64)*64 + p%64
            # via 4 tiny PE transposes of [1,128] slices -> [128,1] columns
            alpha_ps = ps.tile([128, 4], F32, tag="ps")
            for j in range(4):
                nc.tensor.transpose(
                    alpha_ps[:, j : j + 1],
                    nrm[0:1, 0].rearrange("p h d -> p (h d)")[:, ts(j, 128)],
                    id1_f32,
                )
            alpha = small.tile([128, 4], F32, tag="alpha")
            nc.vector.tensor_copy(out=alpha, in_=alpha_ps)

            # inv_k broadcast, pair-packed: partitions 0-63 get even heads,
            # 64-127 get odd heads (two K=1 ones-matmuls)
            ikb_ps = ps.tile([128, 4, D], F32, tag="ps")
            nrm_k = nrm[:, 1]  # [1, H, D]
            nc.tensor.matmul(
                ikb_ps[0:64], ones_f32[:, 0:64], nrm_k[:, 0::2, :],
                start=True, stop=True,
            )
            nc.tensor.matmul(
                ikb_ps[64:128], ones_f32[:, 0:64], nrm_k[:, 1::2, :],
                start=True, stop=True,
            )
            ikb = small.tile([128, 4, D], F32, tag="ikb")
            nc.vector.tensor_copy(out=ikb, in_=ikb_ps)

            # z = gram * inv_k (free axis) * alpha (per partition+slot)
            nc.vector.tensor_mul(out=attn, in0=attn, in1=ikb)
            alpha_bc = bass.AP(
                tensor=alpha.tensor,
                offset=alpha.offset,
                ap=[list(alpha.ap[0]), list(alpha.ap[1]), [0, D]],
            )
            nc.vector.tensor_tensor(
                out=attn, in0=attn, in1=alpha_bc, op=mybir.AluOpType.mult
            )

            # softmax over the last axis (per head)
            mx = small.tile([128, 4], F32, tag="mx")
            nc.vector.tensor_reduce(
                out=mx, in_=attn, axis=mybir.AxisListType.X,
                op=mybir.AluOpType.max, negate=True,
            )
            mx_bc = bass.AP(
                tensor=mx.tensor, offset=mx.offset,
                ap=[list(mx.ap[0]), list(mx.ap[1]), [0, D]],
            )
            nc.vector.tensor_tensor(
                out=attn, in0=attn, in1=mx_bc, op=mybir.AluOpType.add
            )
            ex = small.tile([128, 4, D], F32, tag="ex")
            nc.scalar.activation(
                out=ex, in_=attn, func=mybir.ActivationFunctionType.Exp
            )
            rs = small.tile([128, 4], F32, tag="rs")
            nc.vector.tensor_reduce(
                out=rs, in_=ex, axis=mybir.AxisListType.X, op=mybir.AluOpType.add
            )
            nc.vector.reciprocal(out=rs, in_=rs)
            probs = small.tile([128, 4, D], op_dt, tag="probs")
            rs_bc = bass.AP(
                tensor=rs.tensor, offset=rs.offset,
                ap=[list(rs.ap[0]), list(rs.ap[1]), [0, D]],
            )
            nc.vector.tensor_tensor(
                out=probs, in0=ex, in1=rs_bc, op=mybir.AluOpType.mult
            )

            # transpose probs (per head) -> attnT, same pair-packed layout
            atp = ps.tile([128, 4, D], op_dt, tag="ps")
            for h in range(H):
                r = h % 2
                sl = slice(r * 64, r * 64 + 64)
                nc.tensor.transpose(
                    atp[sl, h // 2, :],
                    probs[sl, h // 2, :],
                    ident[sl, sl],
                )
            attnT = small.tile([128, 4, D], op_dt, tag="attnT")
            nc.vector.tensor_copy(out=attnT, in_=atp)

            # out = attn @ v (channel-major), then proj back to token-major
            for nj in range(N // 512):
                # separate per-group tiles so each proj matmul only waits on
                # the one outT group it actually reads
                outT = [
                    outp.tile([128, 512], op_dt, tag=f"outT{g}", name=f"outT{g}")
                    for g in range(4)
                ]
                for g in range(4):
                    ops = ps.tile([128, 512], F32, tag="ps")
                    for r in range(2):
                        sl = slice(r * 64, r * 64 + 64)
                        nc.tensor.matmul(
                            ops[sl, :],
                            mm(attnT[sl, g, :]),
                            mm(vT[g][sl, ts(nj, 512)]),
                            start=True,
                            stop=True,
                        )
                    nc.scalar.copy(out=outT[g], in_=ops)
                for t4 in range(4):
                    ypt = ps.tile([128, 512], F32, tag="ps")
                    for kc in range(4):
                        nc.tensor.matmul(
                            ypt,
                            mm(outT[kc][:, ts(t4, 128)]),
                            mm(proj_wT[kc]),
                            start=(kc == 0),
                            stop=(kc == 3),
                        )
                    ysb = yp.tile([128, C], F32, tag="ysb")
                    nc.vector.tensor_add(out=ysb, in0=ypt, in1=bias_bc)
                    nc.sync.dma_start(
                        out=out[b, nj * 512 + t4 * 128 : nj * 512 + (t4 + 1) * 128, :],
                        in_=ysb,
                    )

        accp.release()
        ps.release()
        yp.release()
        outp.release()
        small.release()
        vtp.release()
        chunk.release()
        xin.release()
        consts.release()

    legalize_waits(nc)
    return nc


def build_trivial_bass():
    """Minimal kernel used by the benchmark harness to measure the
    per-dispatch floor (axon round trip + runtime overhead)."""
    nc = bass.Bass(trn_type="TRN2")
    inp = nc.dram_tensor("inp", [128, 512], F32, kind="ExternalInput")
    outp = nc.dram_tensor("outp", [128, 512], F32, kind="ExternalOutput")
    with TileContext(nc) as tc:
        with tc.tile_pool(name="p", bufs=1) as pool:
            s = pool.tile([128, 512], F32)
            nc.sync.dma_start(out=s, in_=inp[:, :])
            nc.sync.dma_start(out=outp[:, :], in_=s)
    legalize_waits(nc)
    return nc


_NC_CACHE = {}


def kernel(x, qkv_w, temperature, proj_w, proj_b, _want_trace=False, _trace_kwargs=None):
    x = np.ascontiguousarray(x, dtype=np.float32)
    key = MM_MODE
    if key not in _NC_CACHE:
        _NC_CACHE[key] = build_bass()
    nc = _NC_CACHE[key]

    temp_flat = np.ascontiguousarray(np.asarray(temperature, np.float32).reshape(H))
    if MM_MODE == "bf16":
        import ml_dtypes

        w_dt = ml_dtypes.bfloat16
    else:
        w_dt = np.float32
    qkv_wt = np.ascontiguousarray(np.asarray(qkv_w, np.float32).T.astype(w_dt))
    proj_wt = np.ascontiguousarray(np.asarray(proj_w, np.float32).T.astype(w_dt))
    in_maps = []
    for i in range(NCORES):
        in_maps.append(
            {
                "x": np.ascontiguousarray(x[i * BL : (i + 1) * BL]),
                "qkv_wt": qkv_wt,
                "temperature": temp_flat,
                "proj_wt": proj_wt,
                "proj_b": np.ascontiguousarray(proj_b, np.float32),
            }
        )
    res = run_bass_kernel_spmd(
        nc,
        in_maps,
        core_ids=list(range(NCORES)),
        trace=_want_trace,
        **(_trace_kwargs or {}),
    )
    y = np.concatenate([res.results[i]["out"] for i in range(NCORES)], axis=0)
    if _want_trace:
        return y, res
    return y


# revision 40
# speedup vs baseline: 9.9377x; 1.0087x over previous
"""Trainium2 Bass kernel for nn_CABlock (channel attention / XCA block).

Reference computation (per batch b):
  qkv = x @ qkv_w.T                      # [N, 3C], token-major
  q,k,v per head: [d=64, N] channel-major after reshape/transpose
  q,k l2-normalized over N; attn = softmax((q @ k.T) * temperature, axis=-1)
  out = attn @ v  -> [N, C];  y = out @ proj_w.T + proj_b

Key algebraic restructure: l2norm commutes with the bilinear form, so
  attn_logits = diag(inv_q) @ (q_raw @ k_raw.T) @ diag(inv_k) * temp
with inv_q[c] = 1/max(||q[c,:]||,eps). We accumulate q^T k Gram blocks and
per-channel sums of squares (via a ones-vector matmul over squared values)
in PSUM across all token chunks, then apply the tiny [64x64]-per-head
softmax at the end. This avoids ever materializing normalized q/k.

Sharding: data-parallel over batch B=16 across 8 cores (2 batches/core).
No collectives needed.
"""

import os
import sys

import numpy as np

for _p in ("/opt/trn_rl_repo", "/root/.axon_site/_ro/trn_rl_repo"):
    if os.path.isdir(_p) and _p not in sys.path:
        sys.path.insert(0, _p)

import concourse.bass as bass  # noqa: E402
from concourse import mybir  # noqa: E402
from concourse.bass import ts  # noqa: E402
from concourse.bass_utils import run_bass_kernel_spmd  # noqa: E402
from concourse.masks import make_identity  # noqa: E402
from concourse.tile import TileContext  # noqa: E402

B, N, C = 16, 4096, 512
H, D = 8, 64
C3 = 3 * C
NCORES = 8
BL = B // NCORES  # batches per core
EPS = 1e-12
NCHUNK = N // 128  # 32 token chunks per batch
F32 = mybir.dt.float32

# Matmul operand mode: "bf16" | "fp32" | "f32r" (set BASS_MM_MODE to override)
MM_MODE = os.environ.get("BASS_MM_MODE", "bf16")


def legalize_waits(nc):
    """Walrus in this environment rejects instructions carrying more than one
    semaphore wait ("Too many sync wait commands"), and rejects sem-ge waits
    on Drain instructions entirely. Tile emits both. Hoist the offending
    waits onto standalone EventSemaphore instructions inserted immediately
    before the instruction on the same engine queue — semantically identical
    (the engine executes the waits, then the instruction)."""
    n_new = 0
    for bb in nc.main_func.blocks:
        il = bb.instructions
        new_list = []
        for ins in il:
            si = ins.sync_info
            waits = list(si.on_wait) if si is not None and si.on_wait else []
            if waits:
                tname = type(ins).__name__
                no_wait_slots = tname in ("InstDrain", "InstDmaTransposeAnt") or (
                    getattr(ins, "opcode", "") in ("Drain", "DmaTransposeAnt")
                )
                keep_budget = 0 if no_wait_slots else 1
                if len(waits) > keep_budget:
                    hoist, keep = waits[:-keep_budget] if keep_budget else waits, (
                        waits[-keep_budget:] if keep_budget else []
                    )
                    for w in hoist:
                        ev = mybir.InstEventSemaphore(
                            name=f"{ins.name}-hoistw{n_new}",
                            ins=[],
                            outs=[],
                            engine=ins.engine,
                            sync_info=mybir.SyncInfo(on_wait=[w], on_update=[]),
                        )
                        new_list.append(ev)
                        n_new += 1
                    ins.sync_info = mybir.SyncInfo(
                        on_wait=keep, on_update=list(si.on_update or [])
                    )
            new_list.append(ins)
        il.clear()
        il.extend(new_list)
    return n_new


def build_bass():
    mode = MM_MODE
    op_dt = mybir.dt.bfloat16 if mode == "bf16" else F32

    def mm(ap):
        """Cast an operand AP at a matmul call site for the big matmuls."""
        if mode == "f32r":
            return ap.bitcast(mybir.dt.float32r)
        return ap

    nc = bass.Bass(trn_type="TRN2")
    x = nc.dram_tensor("x", [BL, N, C], F32, kind="ExternalInput")
    # weights arrive pre-transposed ([cin, cout]) and pre-converted to the
    # matmul dtype by the host wrapper
    qkv_wt = nc.dram_tensor("qkv_wt", [C, C3], op_dt, kind="ExternalInput")
    temp = nc.dram_tensor("temperature", [H], F32, kind="ExternalInput")
    proj_wt = nc.dram_tensor("proj_wt", [C, C], op_dt, kind="ExternalInput")
    proj_b = nc.dram_tensor("proj_b", [C], F32, kind="ExternalInput")
    out = nc.dram_tensor("out", [BL, N, C], F32, kind="ExternalOutput")

    with TileContext(nc) as tc:
        consts = tc.alloc_tile_pool(name="consts", bufs=1)
        xin = tc.alloc_tile_pool(name="xin", bufs=6)
        chunk = tc.alloc_tile_pool(name="chunk", bufs=3)
        # fp32 modes double the vT footprint; drop cross-batch double-buffering
        vtp = tc.alloc_tile_pool(name="vtp", bufs=8 if op_dt != F32 else 4)
        small = tc.alloc_tile_pool(name="small", bufs=2)
        outp = tc.alloc_tile_pool(name="outp", bufs=2)
        yp = tc.alloc_tile_pool(name="yp", bufs=3)
        ps = tc.alloc_tile_pool(name="ps", bufs=5, space="PSUM")
        accp = tc.alloc_tile_pool(name="accp", bufs=1, space="PSUM")

        # ---- constants ----
        ident = consts.tile([128, 128], op_dt)
        make_identity(nc, ident)
        ones_col = consts.tile([128, 1], op_dt)
        nc.vector.memset(ones_col, 1.0)
        ones_f32 = consts.tile([1, 128], F32)
        nc.vector.memset(ones_f32, 1.0)
        id1_f32 = consts.tile([1, 1], F32)
        nc.vector.memset(id1_f32, 1.0)

        # temperature: [1, H] row (broadcast along D later via 0-step APs)
        temp_sb = consts.tile([1, H], F32)
        nc.sync.dma_start(out=temp_sb, in_=temp[:])

        # proj bias: load [1, C] then broadcast to all partitions via a
        # K=1 ones-matmul on the PE (out[p, c] = 1 * bias[c])
        bias_row = consts.tile([1, C], F32)
        nc.sync.dma_start(out=bias_row, in_=proj_b[:])
        bias_ps = ps.tile([128, C], F32, tag="ps")
        nc.tensor.matmul(bias_ps, ones_f32, bias_row, start=True, stop=True)
        bias_bc = consts.tile([128, C], F32)
        nc.vector.tensor_copy(out=bias_bc, in_=bias_ps)

        # ---- weights: already [cin, cout] in matmul dtype; plain DMA loads ----
        qkv_wT = [consts.tile([128, C3], op_dt, tag=f"qkvwT{i}", name=f"qkvwT{i}") for i in range(4)]
        proj_wT = [consts.tile([128, C], op_dt, tag=f"projwT{i}", name=f"projwT{i}") for i in range(4)]
        for kc in range(4):
            nc.sync.dma_start(out=qkv_wT[kc], in_=qkv_wt[ts(kc, 128), :])
            nc.sync.dma_start(out=proj_wT[kc], in_=proj_wt[ts(kc, 128), :])

        def phase_a(b):
            # Persistent per-batch PSUM accumulators. Heads are "pair-packed":
            # head h lives at partitions (h%2)*64..(h%2)*64+63.
            # Gram pair-matmuls write [128, 128] blocks per head pair j; the
            # useful data is the diagonal sub-blocks:
            #   acc2[r*64:+64, j, r*64:+64] = sum_n q_h^T k_h  (h = 2j + r)
            acc2 = accp.tile([128, 4, 128], F32, tag="acc")
            ssq_q = accp.tile([1, C], F32, tag="ssq_q")
            ssq_k = accp.tile([1, C], F32, tag="ssq_k")
            # x and v in channel-major layout (tiles per 128-channel group)
            xT = [
                vtp.tile([128, N], op_dt, tag="xt_cm", name=f"xtcm{g}", bufs=4)
                for g in range(4)
            ]
            vT = [vtp.tile([128, N], op_dt, tag="vt", name=f"vt{g}") for g in range(4)]

            # ---------------- Phase A: stream token chunks ----------------
            for ci in range(NCHUNK):
                xt = xin.tile([128, C], F32, tag="xt")
                nc.sync.dma_start(out=xt, in_=x[b, ts(ci, 128), :])
                xb = chunk.tile([128, C], op_dt, tag="xb", bufs=6)
                nc.gpsimd.tensor_copy(out=xb, in_=xt)
                # transpose x chunk into the channel-major batch buffer
                xtp = ps.tile([128, 4, 128], op_dt, tag="ps")
                for g in range(4):
                    nc.tensor.transpose(xtp[:, g, :], xb[:, ts(g, 128)], ident)
                for g in range(4):
                    nc.vector.tensor_copy(out=xT[g][:, ts(ci, 128)], in_=xtp[:, g, :])

                # q,k projection, token-major: stationary = xT chunk
                qp = ps.tile([128, C], F32, tag="ps")
                kp = ps.tile([128, C], F32, tag="ps")
                for kc in range(4):
                    for g, dst in enumerate((qp, kp)):
                        nc.tensor.matmul(
                            dst,
                            mm(xT[kc][:, ts(ci, 128)]),
                            mm(qkv_wT[kc][:, g * C : (g + 1) * C]),
                            start=(kc == 0),
                            stop=(kc == 3),
                        )

                qks = chunk.tile([128, 2, C], op_dt, tag="qks")
                nc.scalar.copy(out=qks[:, 0, :], in_=qp)
                nc.vector.tensor_copy(out=qks[:, 1, :], in_=kp)

                # squared q,k for the sum-of-squares accumulators
                sq = chunk.tile([128, 2, C], op_dt, tag="sq")
                nc.gpsimd.tensor_mul(out=sq, in0=qks, in1=qks)

                # attn Gram accumulation, one [128,128] matmul per head pair
                # (off-diagonal blocks are computed but unused). Two
                # independent accumulation groups per bank (partitions 0-63
                # and 64-127), each with exactly one start and stop.
                for j in range(4):
                    nc.tensor.matmul(
                        acc2[:, j, :],
                        qks[:, 0, ts(j, 2 * D)],
                        qks[:, 1, ts(j, 2 * D)],
                        start=(ci == 0 and j == 0),
                        stop=(ci == NCHUNK - 1 and j == 3),
                    )
                # sum-of-squares via ones-vector matmul
                nc.tensor.matmul(
                    ssq_q,
                    mm(ones_col),
                    mm(sq[:, 0, :]),
                    start=(ci == 0),
                    stop=(ci == NCHUNK - 1),
                )
                nc.tensor.matmul(
                    ssq_k,
                    mm(ones_col),
                    mm(sq[:, 1, :]),
                    start=(ci == 0),
                    stop=(ci == NCHUNK - 1),
                )

                # v projection, channel-major directly: stationary = v weight
                # block [cin, cout128], moving = xT 512-token slices. One
                # (nj, mc) sub-block per chunk to keep PSUM pool pressure flat;
                # group nj's inputs are ready once chunk 4*nj+3 is transposed.
                vjobs = []
                if ci >= 4:
                    vjobs.append((ci // 4 - 1, ci % 4))
                if ci == NCHUNK - 1:
                    vjobs += [(NCHUNK // 4 - 1, mc) for mc in range(4)]
                for nj, mc in vjobs:
                    vps = ps.tile([128, 512], F32, tag="ps")
                    for kc in range(4):
                        nc.tensor.matmul(
                            vps,
                            mm(qkv_wT[kc][:, 2 * C + mc * 128 : 2 * C + (mc + 1) * 128]),
                            mm(xT[kc][:, ts(nj, 512)]),
                            start=(kc == 0),
                            stop=(kc == 3),
                        )
                    nc.scalar.copy(out=vT[mc][:, ts(nj, 512)], in_=vps)

            # extract the PSUM accumulators immediately so the next batch can
            # reuse the accumulator banks while this batch's softmax waits
            attn = small.tile([128, 4, D], F32, tag="attn")
            nc.scalar.copy(out=attn[0:64], in_=acc2[0:64, :, 0:64])
            nc.scalar.copy(out=attn[64:128], in_=acc2[64:128, :, 64:128])
            ssq = small.tile([1, 2, C], F32, tag="ssq")
            nc.vector.tensor_copy(out=ssq[:, 0, :], in_=ssq_q)
            nc.vector.tensor_copy(out=ssq[:, 1, :], in_=ssq_k)
            return attn, ssq, vT

        def phase_b(b, attn, ssq, vT):
            # ---------------- Phase B: softmax + out + proj ----------------
            # Everything pair-packed: [128 partitions, 4 pair slots, 64].

            # inv norm = 1/max(sqrt(ssq), eps); fold temperature into q side
            nrm = small.tile([1, 2, H, D], F32, tag="nrm")
            nc.scalar.sqrt(out=nrm, in_=ssq.rearrange("p t (h d) -> p t h d", h=H))
            nc.vector.tensor_scalar_max(nrm, nrm, EPS)
            nc.vector.reciprocal(out=nrm, in_=nrm)
            temp_bc = bass.AP(
                tensor=temp_sb.tensor,
                offset=temp_sb.offset,
                ap=[list(temp_sb.ap[0]), [1, H], [0, D]],
            )
            nc.vector.tensor_tensor(
                out=nrm[:, 0], in0=nrm[:, 0], in1=temp_bc, op=mybir.AluOpType.mult
            )

            # alpha[p, j] = inv_q[ch] * temp for channel ch=(2j + p//64)*64 + p%64
            # via 4 tiny PE transposes of [1,128] slices -> [128,1] columns
            alpha_ps = ps.tile([128, 4], F32, tag="ps")
            for j in range(4):
                nc.tensor.transpose(
                    alpha_ps[:, j : j + 1],
                    nrm[0:1, 0].rearrange("p h d -> p (h d)")[:, ts(j, 128)],
                    id1_f32,
                )
            alpha = small.tile([128, 4], F32, tag="alpha")
            nc.vector.tensor_copy(out=alpha, in_=alpha_ps)

            # inv_k broadcast, pair-packed: partitions 0-63 get even heads,
            # 64-127 get odd heads (two K=1 ones-matmuls)
            ikb_ps = ps.tile([128, 4, D], F32, tag="ps")
            nrm_k = nrm[:, 1]  # [1, H, D]
            nc.tensor.matmul(
                ikb_ps[0:64], ones_f32[:, 0:64], nrm_k[:, 0::2, :],
                start=True, stop=True,
            )
            nc.tensor.matmul(
                ikb_ps[64:128], ones_f32[:, 0:64], nrm_k[:, 1::2, :],
                start=True, stop=True,
            )
            ikb = small.tile([128, 4, D], F32, tag="ikb")
            nc.vector.tensor_copy(out=ikb, in_=ikb_ps)

            # z = gram * inv_k (free axis) * alpha (per partition+slot)
            nc.vector.tensor_mul(out=attn, in0=attn, in1=ikb)
            alpha_bc = bass.AP(
                tensor=alpha.tensor,
                offset=alpha.offset,
                ap=[list(alpha.ap[0]), list(alpha.ap[1]), [0, D]],
            )
            nc.vector.tensor_tensor(
                out=attn, in0=attn, in1=alpha_bc, op=mybir.AluOpType.mult
            )

            # softmax over the last axis (per head)
            mx = small.tile([128, 4], F32, tag="mx")
            nc.vector.tensor_reduce(
                out=mx, in_=attn, axis=mybir.AxisListType.X,
                op=mybir.AluOpType.max, negate=True,
            )
            mx_bc = bass.AP(
                tensor=mx.tensor, offset=mx.offset,
                ap=[list(mx.ap[0]), list(mx.ap[1]), [0, D]],
            )
            nc.vector.tensor_tensor(
                out=attn, in0=attn, in1=mx_bc, op=mybir.AluOpType.add
            )
            ex = small.tile([128, 4, D], F32, tag="ex")
            nc.scalar.activation(
                out=ex, in_=attn, func=mybir.ActivationFunctionType.Exp
            )
            rs = small.tile([128, 4], F32, tag="rs")
            nc.vector.tensor_reduce(
                out=rs, in_=ex, axis=mybir.AxisListType.X, op=mybir.AluOpType.add
            )
            nc.vector.reciprocal(out=rs, in_=rs)
            probs = small.tile([128, 4, D], op_dt, tag="probs")
            rs_bc = bass.AP(
                tensor=rs.tensor, offset=rs.offset,
                ap=[list(rs.ap[0]), list(rs.ap[1]), [0, D]],
            )
            nc.vector.tensor_tensor(
                out=probs, in0=ex, in1=rs_bc, op=mybir.AluOpType.mult
            )

            # transpose probs (per head) -> attnT, same pair-packed layout
            atp = ps.tile([128, 4, D], op_dt, tag="ps")
            for h in range(H):
                r = h % 2
                sl = slice(r * 64, r * 64 + 64)
                nc.tensor.transpose(
                    atp[sl, h // 2, :],
                    probs[sl, h // 2, :],
                    ident[sl, sl],
                )
            attnT = small.tile([128, 4, D], op_dt, tag="attnT")
            nc.vector.tensor_copy(out=attnT, in_=atp)

            # out = attn @ v (channel-major), then proj back to token-major
            for nj in range(N // 512):
                # separate per-group tiles so each proj matmul only waits on
                # the one outT group it actually reads
                outT = [
                    outp.tile([128, 512], op_dt, tag=f"outT{g}", name=f"outT{g}")
                    for g in range(4)
                ]
                for g in range(4):
                    ops = ps.tile([128, 512], F32, tag="ps")
                    for r in range(2):
                        sl = slice(r * 64, r * 64 + 64)
                        nc.tensor.matmul(
                            ops[sl, :],
                            mm(attnT[sl, g, :]),
                            mm(vT[g][sl, ts(nj, 512)]),
                            start=True,
                            stop=True,
                        )
                    nc.scalar.copy(out=outT[g], in_=ops)
                for t4 in range(4):
                    ypt = ps.tile([128, 512], F32, tag="ps")
                    for kc in range(4):
                        nc.tensor.matmul(
                            ypt,
                            mm(outT[kc][:, ts(t4, 128)]),
                            mm(proj_wT[kc]),
                            start=(kc == 0),
                            stop=(kc == 3),
                        )
                    ysb = yp.tile([128, C], F32, tag="ysb")
                    nc.vector.tensor_add(out=ysb, in0=ypt, in1=bias_bc)
                    nc.sync.dma_start(
                        out=out[b, nj * 512 + t4 * 128 : nj * 512 + (t4 + 1) * 128, :],
                        in_=ysb,
                    )

        # software-pipeline the batches: emit batch b's softmax/out/proj
        # after batch b+1's phase A so the PE queue stays dense while the
        # small softmax chain runs on ACT/DVE
        pending = None
        for b in range(BL):
            ctx = phase_a(b)
            if pending is not None:
                phase_b(*pending)
            pending = (b, *ctx)
        phase_b(*pending)

        accp.release()
        ps.release()
        yp.release()
        outp.release()
        small.release()
        vtp.release()
        chunk.release()
        xin.release()
        consts.release()

    legalize_waits(nc)
    return nc


def build_trivial_bass():
    """Minimal kernel used by the benchmark harness to measure the
    per-dispatch floor (axon round trip + runtime overhead)."""
    nc = bass.Bass(trn_type="TRN2")
    inp = nc.dram_tensor("inp", [128, 512], F32, kind="ExternalInput")
    outp = nc.dram_tensor("outp", [128, 512], F32, kind="ExternalOutput")
    with TileContext(nc) as tc:
        with tc.tile_pool(name="p", bufs=1) as pool:
            s = pool.tile([128, 512], F32)
            nc.sync.dma_start(out=s, in_=inp[:, :])
            nc.sync.dma_start(out=outp[:, :], in_=s)
    legalize_waits(nc)
    return nc


_NC_CACHE = {}


def kernel(x, qkv_w, temperature, proj_w, proj_b, _want_trace=False, _trace_kwargs=None):
    x = np.ascontiguousarray(x, dtype=np.float32)
    key = MM_MODE
    if key not in _NC_CACHE:
        _NC_CACHE[key] = build_bass()
    nc = _NC_CACHE[key]

    temp_flat = np.ascontiguousarray(np.asarray(temperature, np.float32).reshape(H))
    if MM_MODE == "bf16":
        import ml_dtypes

        w_dt = ml_dtypes.bfloat16
    else:
        w_dt = np.float32
    qkv_wt = np.ascontiguousarray(np.asarray(qkv_w, np.float32).T.astype(w_dt))
    proj_wt = np.ascontiguousarray(np.asarray(proj_w, np.float32).T.astype(w_dt))
    in_maps = []
    for i in range(NCORES):
        in_maps.append(
            {
                "x": np.ascontiguousarray(x[i * BL : (i + 1) * BL]),
                "qkv_wt": qkv_wt,
                "temperature": temp_flat,
                "proj_wt": proj_wt,
                "proj_b": np.ascontiguousarray(proj_b, np.float32),
            }
        )
    res = run_bass_kernel_spmd(
        nc,
        in_maps,
        core_ids=list(range(NCORES)),
        trace=_want_trace,
        **(_trace_kwargs or {}),
    )
    y = np.concatenate([res.results[i]["out"] for i in range(NCORES)], axis=0)
    if _want_trace:
        return y, res
    return y
